# revision 7
# baseline (speedup 1.0000x reference)
"""Trainium2 Bass kernel v2 for nn_Decoder (teacher-forced LSTM decoder w/ attention).

Data-parallel over batch N=256 across 8 NeuronCores (32 batch/core), feature-major
layout, 300-step recurrence in a For_i loop. Changes vs v1:

- No Sigmoid anywhere: sigmoid(x) = (tanh(x/2)+1)/2 with doubled cell/hidden
  states (D=2c, H=2h) and host-side weight prescaling. All Act-engine ops are
  {Tanh, Exp} which live in ONE activation table (exp_and_others) -> no
  1283ns table reloads between LSTM pointwise and softmax.
- Gate order (i,f,o,g) so the three half-angle tanhs are one contiguous op.
- Direct onehot x-injection: gates1 x-part computed per step from the M1 table
  (M1 = emb@Wx^T+b1, [V,2048]) against a small per-iteration onehot slab. No
  XG precompute phase, no 38MB HBM round trip.
- Softmax: no max subtraction (energies empirically bounded << 80), masked
  exp + denominator in ONE fused DVE op (scalar_tensor_tensor accum), garbage
  rows get a 1e-30 epsilon via the mask constants, normalization folded into
  the attnT selection matmul (sel columns scaled by 1/sum on DVE).
- ctx PSUM->SBUF copies packed 2-groups-per-op and moved to the Pool engine;
  pred copy on Pool as well.
- Energy PSUM banks triple-buffered (EB0-2) for deeper cross-group pipelining.
"""
import numpy as np
import ml_dtypes

import concourse.bass as bass
import concourse.bacc as bacc
import concourse.tile as tile
from concourse import mybir
from concourse import bass_utils
from concourse._compat import with_exitstack
from contextlib import ExitStack

BF16 = mybir.dt.bfloat16
F32 = mybir.dt.float32
bf16 = ml_dtypes.bfloat16

V, H, KS, VS = 35, 512, 256, 256
NB, T, MAXLEN = 256, 512, 300
NCORES = 8
B = 32            # batch per core
NG = 8            # groups of 4 per core
import os as _os
U = int(_os.environ.get("K_U", "30"))   # steps per For_i iteration
NITER = MAXLEN // U
assert NITER * U == MAXLEN
USE_MAX = False   # subtract row max before exp (off: energies are bounded)
GATES_SPLIT = bool(int(_os.environ.get("K3_SPLIT", "1")))
OH_PRELOAD = bool(int(_os.environ.get("K_OHPRE", "1")))
REP = int(_os.environ.get("K_REP", "1"))   # bench-only: repeat the whole run on-device
FUSE_DIV = bool(int(_os.environ.get("K_FDIV", "0")))  # DVE divide: rejected by walrus ISA check
# timing-ablation levels (numerics only valid at 0):
# 1: no softmax chain (exp/stt/recip/seld skipped, const sel)
# 2: no attention at all (+no outproj)
# 3: also no gates2/pw2
# 4: also no gates1/pw1 (loop + DMA skeleton only)
LVL = int(_os.environ.get("K3_LVL", "0"))
ILV = bool(int(_os.environ.get("K_ILV", "0")))  # interleave col-groups in attention

# PyTorch gate order i,f,g,o -> ours i,f,o,g (rows of the 4*Hc gate matrices)
def _reorder_gates(W, Hc):
    # W: [4*Hc, ...] or [4*Hc]
    i, f, g, o = np.split(W, 4, axis=0)
    return np.concatenate([i, f, o, g], axis=0)


# ----------------------------------------------------------------------------
# host-side planning
# ----------------------------------------------------------------------------
class Plan:
    def __init__(self, lens8):
        lens8 = np.clip(np.asarray(lens8, dtype=np.int64), 1, T - 1)
        order = np.argsort(-lens8, kind="stable")
        # slot s of every core gets one global group of 4; core c gets group
        # order[(s*8+c)*4 : +4]. Padded slot length = longest in slot.
        self.perm = np.zeros((NCORES, B), dtype=np.int64)
        self.Lhat = np.zeros(NG, dtype=np.int64)
        for s in range(NG):
            for c in range(NCORES):
                g = order[(s * NCORES + c) * 4:(s * NCORES + c) * 4 + 4]
                self.perm[c, 4 * s:4 * s + 4] = g
        for s in range(NG):
            self.Lhat[s] = int(lens8[self.perm[:, 4 * s:4 * s + 4]].max())
        self.Tc = np.maximum(1, np.ceil(self.Lhat / 128).astype(np.int64))
        self.lens8 = lens8

        # consts column map (all bf16, [128, CC])
        off = 0
        def take(n):
            nonlocal off
            o = off
            off += int(n)
            return o
        self.m1_o = take(16 * 128)
        self.sel_o = take(NG * B)
        self.mask_o = take(NG * 512)
        self.wc_o = take(2 * 16 * 128)
        self.whh1_o = take(4 * 16 * 128)
        self.wih2_o = take(4 * 8 * 128)
        self.whh2_o = take(2 * 8 * 128)
        self.wout_o = take(4 * V)
        self.b2_o = take(8 * 128)
        self.ones_o = take(B)
        self.kt_o = []
        for b in range(B):
            s = b // 4
            self.kt_o.append(take(2 * self.Lhat[s]))
        self.vt_o = []
        for b in range(B):
            s = b // 4
            self.vt_o.append(take(self.Tc[s] * VS))
        self.cc = off


def build_onehot(plan, core, text):
    """[NITER, V, U*B] bf16: onehot of the teacher tokens, step-major."""
    text = np.asarray(text, np.int64)
    tok = text[plan.perm[core], :MAXLEN]          # [B, MAXLEN]
    oh = np.zeros((NITER, V, U * B), dtype=np.float32)
    for b in range(B):
        for t in range(MAXLEN):
            it, u = divmod(t, U)
            oh[it, tok[b, t], u * B + b] = 1.0
    return oh.astype(bf16)


def build_consts(plan, core, inp):
    """Build the packed [128, cc] bf16 consts array for one core."""
    cc = plan.cc
    A = np.zeros((128, cc), dtype=np.float32)
    perm = plan.perm[core]

    emb = np.asarray(inp["emb"], np.float32)
    W_ih1 = _reorder_gates(np.asarray(inp["W_ih1"], np.float32), H)
    W_hh1 = _reorder_gates(np.asarray(inp["W_hh1"], np.float32), H)
    W_ih2 = _reorder_gates(np.asarray(inp["W_ih2"], np.float32), KS)
    W_hh2 = _reorder_gates(np.asarray(inp["W_hh2"], np.float32), KS)
    W_out = np.asarray(inp["W_out"], np.float32)
    b1 = _reorder_gates(np.asarray(inp["b_ih1"], np.float32)
                        + np.asarray(inp["b_hh1"], np.float32), H)
    b2 = _reorder_gates(np.asarray(inp["b_ih2"], np.float32)
                        + np.asarray(inp["b_hh2"], np.float32), KS)
    enc_key = np.asarray(inp["enc_key"], np.float32)
    enc_values = np.asarray(inp["enc_values"], np.float32)

    # g-gate rows doubled so one tanh(x/2) op serves all four gates:
    # tanh(0.5*(2*g_pre)) == tanh(g_pre)
    W_ih1[3 * H:] *= 2.0; W_hh1[3 * H:] *= 2.0; b1[3 * H:] *= 2.0
    W_ih2[3 * KS:] *= 2.0; W_hh2[3 * KS:] *= 2.0; b2[3 * KS:] *= 2.0

    # states are stored doubled (H=2h): halve every weight that consumes h
    W_hh1 = W_hh1 * 0.5
    W_ih2 = W_ih2 * 0.5
    W_hh2 = W_hh2 * 0.5
    W_out = W_out.copy()
    W_out[:, :KS] *= 0.5          # h2 part; ctx part stays
    enc_key = enc_key * 0.5       # energy contracts with H2=2*h2

    # M1[v, 2048] = emb @ Wx^T + b1  (bias folded; onehot rows sum to 1)
    M1 = emb @ W_ih1[:, :H].T + b1[None, :]
    A[0:V, plan.m1_o:plan.m1_o + 16 * 128] = M1

    # Sel_g[32j, 4g+j] = 1
    sel = A[:, plan.sel_o:plan.sel_o + NG * B].reshape(128, NG, B)
    for g in range(NG):
        for j in range(4):
            sel[32 * j, g, 4 * g + j] = 1.0

    # mask rows 32j: 1 for t < L_n.  Row 1 (always a garbage row) gets a tiny
    # epsilon at t=0 so the fused masked-exp accumulator never sums to exactly
    # zero on garbage rows (keeps 1/sum finite there; sel columns are zero so
    # the value never propagates).
    mask = A[:, plan.mask_o:plan.mask_o + NG * 512].reshape(128, NG, 512)
    for g in range(NG):
        for j in range(4):
            Ln = int(plan.lens8[perm[4 * g + j]])
            mask[32 * j, g, :Ln] = 1.0
        for p in range(128):
            if p % 32 != 0:
                mask[p, g, 0] = 1e-30

    # weights, transposed feature-major: [p, kc, mj]
    wc = A[:, plan.wc_o:plan.wc_o + 2 * 16 * 128].reshape(128, 2, 2048)
    for kc in range(2):
        wc[:, kc, :] = W_ih1[:, H + kc * 128:H + (kc + 1) * 128].T
    whh1 = A[:, plan.whh1_o:plan.whh1_o + 4 * 2048].reshape(128, 4, 2048)
    for kc in range(4):
        whh1[:, kc, :] = W_hh1[:, kc * 128:(kc + 1) * 128].T
    wih2 = A[:, plan.wih2_o:plan.wih2_o + 4 * 1024].reshape(128, 4, 1024)
    for kc in range(4):
        wih2[:, kc, :] = W_ih2[:, kc * 128:(kc + 1) * 128].T
    whh2 = A[:, plan.whh2_o:plan.whh2_o + 2 * 1024].reshape(128, 2, 1024)
    for kc in range(2):
        whh2[:, kc, :] = W_hh2[:, kc * 128:(kc + 1) * 128].T
    wout = A[:, plan.wout_o:plan.wout_o + 4 * V].reshape(128, 4, V)
    for kc in range(4):
        wout[:, kc, :] = W_out[:, kc * 128:(kc + 1) * 128].T

    A[0, plan.b2_o:plan.b2_o + 8 * 128] = b2
    A[0, plan.ones_o:plan.ones_o + B] = 1.0

    for b in range(B):
        s = b // 4
        L = int(plan.Lhat[s])
        n = perm[b]
        kt = A[:, plan.kt_o[b]:plan.kt_o[b] + 2 * L].reshape(128, 2, L)
        for kc in range(2):
            kt[:, kc, :] = enc_key[n, :L, kc * 128:(kc + 1) * 128].T
        Tc = int(plan.Tc[s])
        vt = A[:, plan.vt_o[b]:plan.vt_o[b] + Tc * VS].reshape(128, Tc, VS)
        for tc in range(Tc):
            t0 = tc * 128
            t1 = min(t0 + 128, T)
            vt[0:t1 - t0, tc, :] = enc_values[n, t0:t1, :]
    return A.astype(bf16)


# ----------------------------------------------------------------------------
# program builder
# ----------------------------------------------------------------------------
@with_exitstack
def decoder_kernel(ctx: ExitStack, tc_: tile.TileContext, plan: Plan,
                   consts_h, onehot_h, preds_h, b2_nonzero: bool,
                   niter: int = NITER, dbg_h=None):
    nc = tc_.nc
    cc = plan.cc
    AF = mybir.ActivationFunctionType
    ALU = mybir.AluOpType

    sb = ctx.enter_context(tc_.tile_pool(name="sb", bufs=1))
    pps = ctx.enter_context(tc_.tile_pool(name="pps", bufs=1, space="PSUM"))

    C = sb.tile([128, cc], BF16)
    nc.sync.dma_start(out=C, in_=consts_h[:, :])

    selv = C[:, plan.sel_o:plan.sel_o + NG * B].rearrange("p (g b) -> p g b", g=NG)
    maskv = C[:, plan.mask_o:plan.mask_o + NG * 512].rearrange("p (g t) -> p g t", g=NG)
    wc = C[:, plan.wc_o:plan.wc_o + 2 * 2048].rearrange("p (k m) -> p k m", k=2)
    whh1 = C[:, plan.whh1_o:plan.whh1_o + 4 * 2048].rearrange("p (k m) -> p k m", k=4)
    wih2 = C[:, plan.wih2_o:plan.wih2_o + 4 * 1024].rearrange("p (k m) -> p k m", k=4)
    whh2 = C[:, plan.whh2_o:plan.whh2_o + 2 * 1024].rearrange("p (k m) -> p k m", k=2)
    wout = C[:, plan.wout_o:plan.wout_o + 4 * V].rearrange("p (k v) -> p k v", k=4)
    b2row = C[:, plan.b2_o:plan.b2_o + 8 * 128].rearrange("p (m x) -> p m x", m=8)
    ones = C[:, plan.ones_o:plan.ones_o + B]

    # persistent PSUM banks
    G1A = pps.tile([128, 512], F32, tag="g1a")
    G1B = pps.tile([128, 512], F32, tag="g1b")
    EB0 = pps.tile([128, 512], F32, tag="eb0")
    EB1 = pps.tile([128, 512], F32, tag="eb1")
    EB2 = pps.tile([128, 512], F32, tag="eb2")
    SH = pps.tile([128, 512], F32, tag="sh")      # g2 | ctxT | outproj | attnT
    CXB0 = pps.tile([128, 512], F32, tag="cxb0")  # ctx pairs (0,1), (4,5)
    CXB1 = pps.tile([128, 512], F32, tag="cxb1")  # ctx pairs (2,3), (6,7)
    G1 = [G1A, G1B]
    EB = [EB0, EB1, EB2]
    NEB = 3
    CXB = [CXB0, CXB1]
    g2ps = SH[:, 0:256]
    ctps = SH[:, 256:320].rearrange("p (k b) -> p k b", k=2)
    opps = SH[0:V, 320:352]
    AT = SH[:, 352:480]                           # attnT [128, tc<=4, 32]

    # persistent sbuf state (H1=2*h1, D1=2*c1, H2=2*h2, D2=2*c2)
    h1T = sb.tile([128, 4, B], BF16, tag="h1T")
    d1 = sb.tile([128, 128], F32, tag="d1")
    h2T = sb.tile([128, 2, B], BF16, tag="h2T")
    d2 = sb.tile([128, 64], F32, tag="d2")
    ctxT = sb.tile([128, 2, B], BF16, tag="ctxT")
    attnT = sb.tile([128, 4, B], BF16, tag="attnT")
    exps = sb.tile([128, NG, 512], BF16, tag="exps")
    attns = sb.tile([128, NG, 512], BF16, tag="attns")
    seld = sb.tile([128, NG, B], BF16, tag="seld")
    cxs = sb.tile([128, NG, 256], BF16, tag="cxs")
    tio1 = sb.tile([128, 512], F32, tag="tio1")
    tc1 = sb.tile([128, 128], F32, tag="tc1")
    t1a = sb.tile([128, 128], F32, tag="t1a")
    t1b = sb.tile([128, 128], F32, tag="t1b")
    tio2 = sb.tile([128, 256], F32, tag="tio2")
    tc2 = sb.tile([128, 64], F32, tag="tc2")
    t2a = sb.tile([128, 64], F32, tag="t2a")
    t2b = sb.tile([128, 64], F32, tag="t2b")
    mx = sb.tile([128, NG], F32, tag="mx")
    ssum = sb.tile([128, NG], F32, tag="ssum")
    rsum = sb.tile([128, NG], F32, tag="rsum")

    # prologue: zero states + psum garbage rows + stale-read tiles
    for t in (h1T, h2T, ctxT):
        nc.vector.memset(t, 0.0)
    for t in (d1, d2):
        nc.vector.memset(t, 0.0)
    nc.vector.memset(attns, 0.0)
    nc.vector.memset(EB0, 0.0)
    nc.vector.memset(EB1, 0.0)
    nc.vector.memset(EB2, 0.0)
    nc.vector.memset(CXB0, 0.0)
    nc.vector.memset(CXB1, 0.0)
    # prime the activation table (exp_and_others: Exp+Tanh+Copy) so the
    # per-iteration table reload hoists out of the loop
    nc.scalar.activation(mx[:, 0:1], d1[:, 0:1], AF.Exp)

    ohpool = ctx.enter_context(tc_.tile_pool(name="ohpool", bufs=2))
    prpool = ctx.enter_context(tc_.tile_pool(name="prpool", bufs=2))
    ohall = None
    if OH_PRELOAD:
        ohall = sb.tile([128, MAXLEN * B], BF16, tag="ohall")
        nc.sync.dma_start(
            out=ohall[0:V, :].rearrange("v (o x) -> v o x", o=NITER),
            in_=onehot_h.rearrange("o v x -> v o x"))

    # ---------------- main loop --------------------------------------------
    lhat = [int(x) for x in plan.Lhat]
    tcs = [int(x) for x in plan.Tc]
    TCMAX = max(tcs)
    m1 = C[0:V, plan.m1_o:plan.m1_o + 2048].rearrange("p (m x) -> p m x", m=16)

    def emit_gates_full(u, ohv):
        # whole gates1 for step u: per chunk m a time-consecutive accumulation
        # group (PSUM groups must not interleave with other matmuls)
        g1 = G1[u % 2]
        for m in range(16):
            reg = g1[:, m * 32:(m + 1) * 32]
            nc.tensor.matmul(reg, m1[:, m, :], ohv,
                             start=True, stop=False, skip_group_check=True)
            for kc in range(2):
                nc.tensor.matmul(reg, wc[:, kc, m * 128:(m + 1) * 128],
                                 ctxT[:, kc, :], start=False, stop=False, skip_group_check=True)
            for kc in range(4):
                nc.tensor.matmul(reg, whh1[:, kc, m * 128:(m + 1) * 128],
                                 h1T[:, kc, :], start=False, stop=(kc == 3), skip_group_check=True)

    def emit_step(u, ohv_cur, ohv_next, predv):
        g1 = G1[u % 2]
        if LVL >= 4:
            nc.vector.memset(predv, 0.0)
            return
        if not GATES_SPLIT:
            emit_gates_full(u, ohv_cur)
        # pointwise 1 (gate cols: i 0:128, f 128:256, o 256:384, g 384:512;
        # g-gate weights pre-doubled so one tanh(x/2) op covers everything)
        nc.scalar.activation(tio1, g1[:, 0:512], AF.Tanh, scale=0.5)
        # D1' = ((tf+1)*D1)*0.5 + (ti+1)*G
        nc.vector.scalar_tensor_tensor(out=t1a, in0=tio1[:, 128:256], scalar=1.0,
                                       in1=d1, op0=ALU.add, op1=ALU.mult)
        nc.vector.scalar_tensor_tensor(out=t1b, in0=tio1[:, 0:128], scalar=1.0,
                                       in1=tio1[:, 384:512], op0=ALU.add, op1=ALU.mult)
        nc.vector.scalar_tensor_tensor(out=d1, in0=t1a, scalar=0.5,
                                       in1=t1b, op0=ALU.mult, op1=ALU.add)
        nc.scalar.activation(tc1, d1, AF.Tanh, scale=0.5)
        nc.vector.scalar_tensor_tensor(out=h1T.rearrange("p a b -> p (a b)"),
                                       in0=tio1[:, 256:384], scalar=1.0,
                                       in1=tc1, op0=ALU.add, op1=ALU.mult)
        if LVL >= 3:
            nc.vector.memset(predv, 0.0)
            if GATES_SPLIT and ohv_next is not None:
                emit_gates_full(u + 1, ohv_next)
            return
        # gates2
        for m in range(8):
            reg = g2ps[:, m * 32:(m + 1) * 32]
            for kc in range(4):
                nc.tensor.matmul(reg, wih2[:, kc, m * 128:(m + 1) * 128],
                                 h1T[:, kc, :], start=(kc == 0), stop=False, skip_group_check=True)
            for kc in range(2):
                last = (kc == 1) and not b2_nonzero
                nc.tensor.matmul(reg, whh2[:, kc, m * 128:(m + 1) * 128],
                                 h2T[:, kc, :], start=False, stop=last, skip_group_check=True)
            if b2_nonzero:
                nc.tensor.matmul(reg, b2row[0:1, m, :], ones[0:1, :],
                                 start=False, stop=True, skip_group_check=True)
        # pointwise 2 (i 0:64, f 64:128, o 128:192, g 192:256)
        nc.scalar.activation(tio2, g2ps[:, 0:256], AF.Tanh, scale=0.5)
        nc.vector.scalar_tensor_tensor(out=t2a, in0=tio2[:, 64:128], scalar=1.0,
                                       in1=d2, op0=ALU.add, op1=ALU.mult)
        nc.vector.scalar_tensor_tensor(out=t2b, in0=tio2[:, 0:64], scalar=1.0,
                                       in1=tio2[:, 192:256], op0=ALU.add, op1=ALU.mult)
        nc.vector.scalar_tensor_tensor(out=d2, in0=t2a, scalar=0.5,
                                       in1=t2b, op0=ALU.mult, op1=ALU.add)
        nc.scalar.activation(tc2, d2, AF.Tanh, scale=0.5)
        nc.vector.scalar_tensor_tensor(out=h2T.rearrange("p a b -> p (a b)"),
                                       in0=tio2[:, 128:192], scalar=1.0,
                                       in1=tc2, op0=ALU.add, op1=ALU.mult)
        if LVL >= 2:
            nc.vector.memset(predv, 0.0)
            if GATES_SPLIT and ohv_next is not None:
                emit_gates_full(u + 1, ohv_next)
            return

        # attention phase E: all energies back-to-back on PE; softmax chains
        # (Act/DVE) drain behind them without blocking the PE queue.
        # K_ILV: interleave col-groups (j inner-to-outer) so consecutive MMs
        # target different 32-col array strips and stream concurrently
        # (matmul starts are pc-monotone; same-strip pairs serialize).
        for g in range(NG):
            L = lhat[g]
            eb = EB[g % NEB]
            if ILV:
                for kc in range(2):
                    for j in range(4):
                        b = 4 * g + j
                        nc.tensor.matmul(
                            eb[32 * j:32 * j + 1, 0:L],
                            h2T[:, kc, b:b + 1],
                            C[:, plan.kt_o[b] + kc * L:plan.kt_o[b] + (kc + 1) * L],
                            start=(kc == 0), stop=(kc == 1),
                            tile_position=(0, 32 * j), skip_group_check=True)
            else:
                for j in range(4):
                    b = 4 * g + j
                    for kc in range(2):
                        nc.tensor.matmul(
                            eb[32 * j:32 * j + 1, 0:L],
                            h2T[:, kc, b:b + 1],
                            C[:, plan.kt_o[b] + kc * L:plan.kt_o[b] + (kc + 1) * L],
                            start=(kc == 0), stop=(kc == 1),
                            tile_position=(0, 32 * j), skip_group_check=True)
            if LVL >= 1:
                continue
            nc.scalar.activation(exps[:, g, 0:L], eb[:, 0:L], AF.Exp)
            # attns = exps*mask (cols beyond L stay zero from prologue memset),
            # ssum = per-row masked denominator, in one DVE op
            nc.vector.scalar_tensor_tensor(out=attns[:, g, 0:L],
                                           in0=exps[:, g, 0:L], scalar=1.0,
                                           in1=maskv[:, g, 0:L],
                                           op0=ALU.mult, op1=ALU.mult,
                                           accum_out=ssum[:, g:g + 1])
        # one reciprocal + one broadcast-multiply for all 8 groups (replaces
        # 16 tiny DVE ops and their semaphore hops); A-phase only starts after
        # the whole E phase anyway, so the extra latency on early groups is free
        nc.vector.reciprocal(rsum[:, 0:NG], ssum[:, 0:NG])
        nc.vector.tensor_mul(
            seld.rearrange("p g b -> p g b"),
            selv,
            rsum[:, 0:NG].unsqueeze(2).broadcast_to([128, NG, B]))
        # phase A: transpose normalized attn rows into AT via scaled selection
        selsrc = seld if LVL == 0 else selv
        for g in range(NG):
            for tcc in range(tcs[g]):
                nc.tensor.matmul(
                    AT[:, tcc * 32:(tcc + 1) * 32],
                    attns[0:97, g, tcc * 128:(tcc + 1) * 128],
                    selsrc[0:97, g, :],
                    start=(g == 0 and tcc == 0), stop=(g == NG - 1),
                    skip_group_check=True)
        nc.scalar.activation(
            attnT.rearrange("p a b -> p (a b)")[:, 0:TCMAX * 32],
            AT[:, 0:TCMAX * 32], AF.Copy)
        # phase C: ctx row-form, pairs alternate between two PSUM banks so the
        # PSUM->SBUF pair-copies overlap the next pair's matmuls
        for g in range(NG):
            cxb = CXB[(g // 2) % 2]
            cxp = cxb[:, (g % 2) * 256:(g % 2) * 256 + 256]
            if ILV:
                for tcc in range(tcs[g]):
                    for j in range(4):
                        b = 4 * g + j
                        nc.tensor.matmul(
                            cxp[32 * j:32 * j + 1, :],
                            attnT[:, tcc, b:b + 1],
                            C[:, plan.vt_o[b] + tcc * VS:plan.vt_o[b] + (tcc + 1) * VS],
                            start=(tcc == 0), stop=(tcc == tcs[g] - 1),
                            tile_position=(0, 32 * j), skip_group_check=True)
            else:
                for j in range(4):
                    b = 4 * g + j
                    for tcc in range(tcs[g]):
                        nc.tensor.matmul(
                            cxp[32 * j:32 * j + 1, :],
                            attnT[:, tcc, b:b + 1],
                            C[:, plan.vt_o[b] + tcc * VS:plan.vt_o[b] + (tcc + 1) * VS],
                            start=(tcc == 0), stop=(tcc == tcs[g] - 1),
                            tile_position=(0, 32 * j), skip_group_check=True)
            if g % 2 == 1:
                if (g // 2) % 2 == 0:
                    nc.vector.tensor_copy(
                        cxs[:, g - 1:g + 1, :].rearrange("p a b -> p (a b)"),
                        cxb[0:128, :])
                else:
                    nc.scalar.activation(
                        cxs[:, g - 1:g + 1, :].rearrange("p a b -> p (a b)"),
                        cxb[0:128, :], AF.Copy)
        # phase T: ctxT via Sel accumulation
        for g in range(NG):
            for vc in range(2):
                nc.tensor.matmul(ctps[:, vc, :],
                                 cxs[0:97, g, vc * 128:(vc + 1) * 128],
                                 selv[0:97, g, :],
                                 start=(g == 0 and vc == 0),
                                 stop=(g == NG - 1 and vc == 1),
                                 skip_group_check=True)
        nc.scalar.activation(ctxT.rearrange("p a b -> p (a b)"),
                             ctps.rearrange("p a b -> p (a b)"), AF.Copy)
        # next step's gates1 first (it gates the next pw1); the output
        # projection follows as a pw1-stall filler — preds feed only the DMA
        if GATES_SPLIT and ohv_next is not None:
            emit_gates_full(u + 1, ohv_next)
        for kc in range(4):
            rhs = h2T[:, kc, :] if kc < 2 else ctxT[:, kc - 2, :]
            nc.tensor.matmul(opps, wout[:, kc, :], rhs,
                             start=(kc == 0), stop=(kc == 3), skip_group_check=True)
        nc.vector.tensor_copy(predv, opps)

    UNROLL = bool(int(_os.environ.get("K_UNROLL", "0")))

    def loop_body(riv):
        iv = riv % niter if REP > 1 else riv
        if OH_PRELOAD:
            def ohv_of(u):
                return ohall[0:V, bass.ds(iv * (U * B) + u * B, B)]
        else:
            ohslab = ohpool.tile([V, U * B], BF16)
            nc.sync.dma_start(out=ohslab,
                              in_=onehot_h[bass.ds(iv, 1)].rearrange("o v x -> v (o x)"))
            def ohv_of(u):
                return ohslab[:, u * B:(u + 1) * B]
        predsb = prpool.tile([V, U, B], F32)
        if GATES_SPLIT:
            emit_gates_full(0, ohv_of(0))
        for u in range(U):
            ohv_next = ohv_of(u + 1) if u + 1 < U else None
            emit_step(u, ohv_of(u), ohv_next, predsb[:, u, :])
            if dbg_h is not None and u == 0:
                dsb = prpool.tile([128, 1408], BF16, tag="dsb")
                nc.vector.tensor_copy(dsb[:, 0:128], h1T.rearrange("p a b -> p (a b)"))
                nc.vector.tensor_copy(dsb[:, 128:192], h2T.rearrange("p a b -> p (a b)"))
                nc.vector.tensor_copy(dsb[:, 192:256], ctxT.rearrange("p a b -> p (a b)"))
                nc.vector.tensor_copy(dsb[:, 256:384], attnT.rearrange("p a b -> p (a b)"))
                nc.vector.tensor_copy(dsb[:, 384:896], attns[:, 0, :])
                nc.vector.tensor_copy(dsb[:, 896:1152], cxs[:, 0, :])
                nc.vector.tensor_copy(dsb[:, 1152:1408], cxs[:, 1, :])
                nc.sync.dma_start(out=dbg_h[bass.ds(iv, 1)].rearrange("o p x -> p (o x)"), in_=dsb)
        nc.sync.dma_start(out=preds_h[bass.ds(iv, 1)].rearrange("o v u b -> v u (o b)"),
                          in_=predsb)

    if UNROLL:
        for riv in range(niter * REP):
            loop_body(riv)
    else:
        with tc_.For_i(0, niter * REP, 1, hint_engines=(
                mybir.EngineType.PE, mybir.EngineType.DVE,
                mybir.EngineType.Activation)) as riv:
            loop_body(riv)


# ----------------------------------------------------------------------------
# entry point
# ----------------------------------------------------------------------------
_CACHE = {}
LAST_EXEC_NS = None
LAST_RESULTS = None


def _build_program(plan, b2_nonzero, niter=NITER, dbg=False):
    nc = bacc.Bacc("TRN2", debug=False)
    consts_h = nc.dram_tensor("consts", [128, plan.cc], BF16, kind="ExternalInput")
    onehot_h = nc.dram_tensor("onehot", [NITER, V, U * B], BF16, kind="ExternalInput")
    preds_h = nc.dram_tensor("preds", [niter, V, U, B], F32, kind="ExternalOutput")
    dbg_h = nc.dram_tensor("dbg", [niter, 128, 1408], BF16, kind="ExternalOutput") if dbg else None
    with tile.TileContext(nc) as tc_:
        decoder_kernel(tc_, plan, consts_h, onehot_h[:, :, :],
                       preds_h, b2_nonzero, niter=niter, dbg_h=dbg_h)
    nc.compile()
    return nc


def kernel(**inp):
    global LAST_EXEC_NS, LAST_RESULTS
    import os
    lens = np.asarray(inp["lens"], np.int64)
    lens8 = lens // 8
    plan = Plan(lens8)
    b2 = np.asarray(inp["b_ih2"], np.float32) + np.asarray(inp["b_hh2"], np.float32)
    b2_nonzero = bool(np.any(b2 != 0.0))

    dbg = bool(int(os.environ.get("KDBG", "0")))
    key = (tuple(plan.Lhat), b2_nonzero, dbg)
    if key not in _CACHE:
        _CACHE[key] = _build_program(plan, b2_nonzero, dbg=dbg)
    nc = _CACHE[key]

    in_maps = []
    for c in range(NCORES):
        A = build_consts(plan, c, inp)
        OH = build_onehot(plan, c, inp["text"])
        in_maps.append({"consts": A, "onehot": OH})
    kw = {}
    if os.environ.get("KTRACE"):
        kw = dict(trace=True, tmpdir=os.environ.get("KTRACE_DIR") or None)
    res = bass_utils.run_bass_kernel_spmd(nc, in_maps, core_ids=list(range(NCORES)), **kw)
    LAST_EXEC_NS = res.exec_time_ns
    LAST_RESULTS = res

    b_out = np.asarray(inp["b_out"], np.float32)
    out = np.zeros((NB, MAXLEN, V), dtype=np.float32)
    for c in range(NCORES):
        p = res.results[c]["preds"]            # [NITER, V, U, B]
        p = np.transpose(p, (3, 0, 2, 1)).reshape(B, MAXLEN, V)
        out[plan.perm[c]] = p
    out += b_out[None, None, :]
    return out

